# revision 6
# baseline (speedup 1.0000x reference)
"""Trainium2 Bass kernel for BF16IndexerBaseline (sparse_attention).

Computes, for q:(1,M,H,D) bf16, k:(1,N,D) bf16, weights:(H,M) bf16:

    index_score[b,m,n] = sum_h relu(q[b,m,h,:] . k[b,n,:]) * (weights[h,m]*D**-0.5)

Strategy (8 NeuronCores, SPMD, host-side sharding of m):
  - each core gets an m-shard of 256 rows (2 m-tiles of 128), k replicated.
  - host-side prep (numpy, part of sharding): q is pre-scaled by
    s[h,m] = bf16(weights*D**-0.5) (valid since s>=0 commutes with relu),
    and both q and k are pre-transposed to [d, .] layout so the device
    does plain DMA loads -- no xbar transposes, no on-device scales.
  - per (m-tile, n-chunk of 1024) unit: 16 heads x 2 matmuls (K=D=128)
    -> fp32 logits in PSUM. PSUM eviction (1 elem/lane/cyc, ACT+DVE only)
    is the bottleneck; split:
      * V heads chained on VectorE via fused custom DVE op
        acc = relu(psum) + acc; the first chain op is seeded with an
        A-side partial (in1=g1) to save a separate combine.
      * A heads relu-evicted to bf16 r-tiles on ScalarE; summed by a tree
        spread over DMA-CCE accumulates (earliest slots, to hide their
        latency), GpSimd tensor_adds, and leftover VectorE bf16 2x adds.
  - final: stage = chain_acc + tree_root, alternating Vector/GpSimd per
    unit; DMA to DRAM on the sync queue.
"""

import os

os.environ.setdefault("MYCRO_LOCAL_CACHE", "1")

import numpy as np
import ml_dtypes
from contextlib import ExitStack

import concourse.bass as bass
import concourse.tile as tile
from concourse import bacc, mybir
from concourse.bass_utils import run_bass_kernel_spmd

# ---------------------------------------------------------------- problem dims
B = 1
M = 2048
H = 16
N = 4096
D = 128
N_CORES = 8
MS = M // N_CORES          # 256 rows of m per core
MT = MS // 128             # 2 m-tiles per core
FD = 1024                  # n-chunk (free dim) per epilogue op = 2 PSUM banks
NCH = N // FD              # 4 n-chunks

V_HEADS = int(os.environ.get("IDX_V", "7"))          # DVE-chained heads
SEED = bool(int(os.environ.get("IDX_SEED", "1")))    # seed chain op0 with g1
CCE_N = int(os.environ.get("IDX_CCE", "3"))          # DMA-CCE pair-accumulates per unit
FINAL_MODE = int(os.environ.get("IDX_FINAL", "2"))   # 0=DVE 1=GPS 2=alternate
TAIL_DVE = bool(int(os.environ.get("IDX_TAILDVE", "1")))  # last unit: DVE tree, no CCE

BF16 = mybir.dt.bfloat16
F32 = mybir.dt.float32
SCALE_BF16 = float(np.float32(np.array(D ** -0.5, dtype=ml_dtypes.bfloat16)))

# --------------------------------------------------- custom fused DVE op
# out = relu(in0 * s0) + in1   (s0 scalar; used with s0=1.0 since q is prescaled)
import concourse.dve_ops as dve_ops
from concourse.dve_spec import Spec as _Spec, Src0 as _Src0, Src1 as _Src1, C0 as _C0
from concourse.dve_spec import relu as _relu, lower as _lower
from concourse.dve_uop import DveOpSpec as _DveOpSpec

_OP_NAME = "RELU_SCALE_ADD_ANT"


def _ref_relu_scale_add(in0, in1, s0, s1, imm2):
    x = np.nan_to_num(in0.astype(np.float32) * s0, nan=0.0, posinf=np.inf, neginf=-np.inf)
    return np.maximum(x, 0.0).astype(np.float32) + in1


def _register_relu_scale_add():
    for op in dve_ops.OPS:
        if op.name == _OP_NAME:
            return op
    spec = _Spec(body=_relu(_Src0 * _C0) + _Src1, reference=_ref_relu_scale_add)
    row = max(dve_ops._SUB_OPCODE_FOR_NAME.values()) + 1
    assert row < 0x20
    dve_ops._SUB_OPCODE_FOR_NAME[_OP_NAME] = row
    shas = {
        v: _DveOpSpec(name=_OP_NAME, opcode=row, uops=_lower(spec, ver=v), rd1_en=True).sha(v)
        for v in ("v3", "v4")
    }
    op = dve_ops.DveOp(_OP_NAME, spec, subdim=False, uops_sha=shas)
    dve_ops.OPS.append(op)
    dve_ops.CUSTOM_DVE_SPECS[_OP_NAME] = spec
    return op


RELU_SCALE_ADD = _register_relu_scale_add()


def _head_roles(v_heads: int) -> list[str]:
    """A/V pattern: V heads interleaved from position 4 (evens), then odd tail."""
    pos = [p for p in range(4, 16, 2)]
    pos += [p for p in range(15, 0, -2) if p not in pos]
    vset = set(pos[:v_heads])
    return ["V" if i in vset else "A" for i in range(16)]


# ------------------------------------------------------------------ kernel IR
def _emit(ctx: ExitStack, tc: "tile.TileContext", q_d, k_d, o_d):
    nc = tc.nc
    AOp = mybir.AluOpType
    roles = _head_roles(V_HEADS)
    n_a = roles.count("A")
    assert 8 <= n_a <= 10, f"tree schedule assumes 8..10 A-heads, got {n_a}"

    const = ctx.enter_context(tc.tile_pool(name="const", bufs=1))
    psA = ctx.enter_context(tc.tile_pool(name="psA", bufs=2, space="PSUM"))
    psV = ctx.enter_context(tc.tile_pool(name="psV", bufs=2, space="PSUM"))
    rpool = ctx.enter_context(tc.tile_pool(name="rpool", bufs=3 * n_a))
    tpool = ctx.enter_context(tc.tile_pool(name="tpool", bufs=6))
    apool = ctx.enter_context(tc.tile_pool(name="apool", bufs=3))
    opool = ctx.enter_context(tc.tile_pool(name="opool", bufs=3))

    # plain piece-wise loads (host already transposed to [d, .])
    # qT pieces: 4 x [128,1024] covering heads 4g..4g+3 (col = h*MS + m_local)
    # kT pieces: 2 x [128,2048] covering n-chunks {0,1} and {2,3}
    qtp = [const.tile([128, 1024], BF16, name=f"qT{g}") for g in range(4)]
    ktp = [const.tile([128, 2048], BF16, name=f"kT{p}") for p in range(2)]
    nc.scalar.dma_start(out=qtp[0][:], in_=q_d[:, 0:1024])
    nc.sync.dma_start(out=ktp[0][:], in_=k_d[:, 0:2048])
    nc.sync.dma_start(out=qtp[1][:], in_=q_d[:, 1024:2048])
    nc.sync.dma_start(out=qtp[2][:], in_=q_d[:, 2048:3072])
    nc.sync.dma_start(out=qtp[3][:], in_=q_d[:, 3072:4096])
    nc.sync.dma_start(out=ktp[1][:], in_=k_d[:, 2048:4096])

    def lhs_ap(h, mt):
        g, hh = h // 4, h % 4
        c0 = hh * MS + mt * 128
        return qtp[g][:, c0: c0 + 128]

    def rhs_ap(nci, j):
        p, pp = nci // 2, nci % 2
        c0 = pp * 1024 + j * 512
        return ktp[p][:, c0: c0 + 512]

    for mt in range(MT):
        for nci in range(NCH):
            uid = f"{mt}_{nci}"
            last_unit = (mt == MT - 1) and (nci == NCH - 1)
            use_cce = 0 if (TAIL_DVE and last_unit) else CCE_N

            acc = apool.tile([128, FD], F32, tag="acc", name=f"acc_{uid}")
            stage = opool.tile([128, FD], F32, tag="stage", name=f"stage_{uid}")
            slots = []           # bf16 r tiles, in eviction order
            g1 = g2 = g3 = None
            chain_i = 0
            dve_partials = []    # tiles DVE must fold before final

            def _evict_a(pt):
                i = len(slots)
                r = rpool.tile([128, FD], BF16, tag="r", name=f"r{i}_{uid}")
                nc.scalar.activation(r[:], pt[:], mybir.ActivationFunctionType.Relu)
                slots.append(r)
                # tree triggers keyed on slot count:
                # g1 = s0+s1 on GPS early (seeds the chain before the first
                # V-head's op runs); CCE pairs on s2..s5; g2 = s6+s7 on GPS;
                # leftovers folded by DVE at unit end.
                if use_cce >= 1 and i == 4:
                    nc.gpsimd.dma_start(out=slots[2][:], in_=slots[4][:], accum_op=AOp.add)
                if use_cce >= 2 and i == 5:
                    nc.gpsimd.dma_start(out=slots[3][:], in_=slots[5][:], accum_op=AOp.add)
                if use_cce >= 3 and i == 5:
                    nc.gpsimd.dma_start(out=slots[2][:], in_=slots[3][:], accum_op=AOp.add)

            def _chain_v(pt):
                nonlocal chain_i
                if chain_i == 0:
                    if SEED and g1 is not None:
                        nc.vector._custom_dve(
                            RELU_SCALE_ADD, out=acc[:], in0=pt[:], in1=g1[:], s0=1.0
                        )
                    else:
                        nc.vector.tensor_scalar(acc[:], pt[:], 0.0, None, op0=AOp.max)
                else:
                    nc.vector._custom_dve(
                        RELU_SCALE_ADD, out=acc[:], in0=pt[:], in1=acc[:], s0=1.0
                    )
                chain_i += 1

            # ---- main head loop
            for h in range(16):
                pool = psV if roles[h] == "V" else psA
                pt = pool.tile([128, FD], F32, tag="logits", name=f"ps_{uid}_{h}")
                for j in range(FD // 512):
                    nc.tensor.matmul(
                        pt[:, j * 512: (j + 1) * 512],
                        lhs_ap(h, mt),
                        rhs_ap(nci, j),
                        start=True,
                        stop=True,
                    )
                if roles[h] == "A":
                    was = len(slots)
                    _evict_a(pt)
                    # GPS tree ops keyed on eviction progress
                    if was + 1 == 2:
                        # g1 = s0+s1 (seeds the chain; fp32 so in1 dtype is safe)
                        g1 = tpool.tile([128, FD], F32 if SEED else BF16,
                                        tag="gf" if SEED else "gb", name=f"g1_{uid}")
                        nc.gpsimd.tensor_add(g1[:], slots[0][:], slots[1][:])
                    if was + 1 == 8 and n_a >= 8:
                        g2 = tpool.tile([128, FD], BF16, tag="gb", name=f"g2_{uid}")
                        nc.gpsimd.tensor_add(g2[:], slots[6][:], slots[7][:])
                else:
                    _chain_v(pt)

            # ---- post-loop tree
            assert len(slots) == n_a
            extra = slots[9:]
            if g2 is not None and n_a >= 9:
                g3 = tpool.tile([128, FD], BF16, tag="gb", name=f"g3_{uid}")
                nc.gpsimd.tensor_add(g3[:], g2[:], slots[8][:])
            elif g2 is not None:
                g3 = g2

            # what remains to fold: CCE-collapsed mid slots + g3 + any extra
            # slots (+ g1 if unseeded)
            rem = []
            if use_cce >= 3:
                rem.append(slots[2])
            elif use_cce == 2:
                rem += [slots[2], slots[3]]
            elif use_cce == 1:
                rem += [slots[2], slots[3], slots[5]]
            else:
                rem += [slots[2], slots[3], slots[4], slots[5]]
            if g3 is not None:
                rem.append(g3)
            rem += extra
            if not SEED and g1 is not None:
                rem.append(g1)

            # fold rem on DVE (bf16 2x adds)
            wi = 0
            while len(rem) > 1:
                t = tpool.tile([128, FD], BF16, tag="u", name=f"u{wi}_{uid}")
                wi += 1
                nc.vector.tensor_add(t[:], rem[0][:], rem[1][:])
                rem = [t] + rem[2:]
            root = rem[0] if rem else None

            # final combine
            if root is not None:
                use_gps = (FINAL_MODE == 1) or (FINAL_MODE == 2 and (mt * NCH + nci) % 2 == 0)
                if last_unit:
                    use_gps = False
                eng = nc.gpsimd if use_gps else nc.vector
                eng.tensor_add(stage[:], acc[:], root[:])
            else:
                nc.vector.tensor_copy(stage[:], acc[:])
            nc.sync.dma_start(
                out=o_d[mt * 128: (mt + 1) * 128, nci * FD: (nci + 1) * FD],
                in_=stage[:],
            )


_NC_CACHE = None


def _build():
    global _NC_CACHE
    if _NC_CACHE is not None:
        return _NC_CACHE
    nc = bacc.Bacc(
        "TRN2",
        target_bir_lowering=False,
        debug=False,
        enable_asserts=False,
        num_devices=N_CORES,
    )
    q_d = nc.dram_tensor("q", [D, H * MS], BF16, kind="ExternalInput").ap()
    k_d = nc.dram_tensor("k", [D, N], BF16, kind="ExternalInput").ap()
    o_d = nc.dram_tensor("o", [MS, N], F32, kind="ExternalOutput").ap()
    with tile.TileContext(nc) as tc:
        with ExitStack() as ctx:
            _emit(ctx, tc, q_d, k_d, o_d)
    nc.compile()
    _NC_CACHE = (nc, q_d, k_d, o_d)
    return _NC_CACHE


def _shard_inputs(q, k, weights):
    bf16 = ml_dtypes.bfloat16
    q = np.asarray(q).astype(bf16, copy=False).reshape(M, H, D)
    k = np.asarray(k).astype(bf16, copy=False).reshape(N, D)
    w = np.asarray(weights).astype(bf16, copy=False).reshape(H, M)
    # s[h,m] = bf16(w * bf16(scale)); prescale q (s >= 0 commutes with relu)
    s = (w * np.asarray(SCALE_BF16, dtype=bf16)).astype(bf16)
    qs = (q * s.T[:, :, None]).astype(bf16)          # (M,H,D) bf16
    kT = np.ascontiguousarray(k.T)                   # (D,N)
    in_maps = []
    for c in range(N_CORES):
        m0 = c * MS
        # qT_c[d, h*MS+m] = qs[m0+m, h, d]
        qT_c = np.ascontiguousarray(
            qs[m0: m0 + MS].transpose(2, 1, 0).reshape(D, H * MS)
        )
        in_maps.append({"q": qT_c, "k": kT})
    return in_maps


LAST_RESULTS = None


def kernel(q, k, weights):
    global LAST_RESULTS
    nc, *_ = _build()
    in_maps = _shard_inputs(q, k, weights)
    trace = bool(int(os.environ.get("IDX_TRACE", "0")))
    res = run_bass_kernel_spmd(
        nc, in_maps, core_ids=list(range(N_CORES)), trace=trace
    )
    LAST_RESULTS = res
    out = np.empty((B, M, N), np.float32)
    for c in range(N_CORES):
        out[0, c * MS: (c + 1) * MS] = res.results[c]["o"]
    return out
